# revision 5
# baseline (speedup 1.0000x reference)
"""Block-circulant SwiGLU feed-forward (CirculantFeedForward) for 8 trn2 cores.

Strategy: token-parallel across the 8 cores (16384 tokens -> 2048/core, no
collectives). Each circulant block is materialized host-side into a dense
matrix, so on-device the whole network is three dense GEMMs + SiLU*mul,
running on the TensorEngine. Host also pre-packs x / weights into the exact
SBUF layouts so every DMA is a contiguous [128, F] transfer.

Per-core geometry (d_model=2048, d_ff=5632, block=512):
  gate/up:  out[m(44x128), tok] += Wgu[k(16x128), m].T @ xT[k, tok]
  h = silu(gate) * up                  (stored bf16/f32 in SBUF)
  down:     out[m2(16x128), tok] += Wd[k2(44x128), m2].T @ h[k2, tok]
Tokens are processed in passes sized to fit SBUF.

PSUM budget (8 banks): two tags "a"/"b" of [128, pass_t] fp32, bufs=2.
gate accumulates in "a", up in "b"; down reuses "a".
"""
import os
from contextlib import ExitStack

import numpy as np
import ml_dtypes

import concourse.bacc as bacc
import concourse.mybir as mybir
import concourse.tile as tile
from concourse.bass_utils import run_bass_kernel_spmd

N_CORES = 8
P = 128
B = 512
D_MODEL = 2048
D_FF = 5632
KT = D_MODEL // P    # 16 k-tiles (gate/up contraction; also down output tiles)
MT = D_FF // P       # 44 m-tiles (gate/up output; down contraction)
TOK_TOTAL = 16384
TOK_CORE = TOK_TOTAL // N_CORES  # 2048

MODE = os.environ.get("BASS_MODE", "bf16")  # bf16 | f32 | f32r

_MODE_CFG = {
    # (mybir dtype, numpy dtype, tokens per pass, matmul N)
    "bf16": (mybir.dt.bfloat16, ml_dtypes.bfloat16, 1024, 512),
    "f32":  (mybir.dt.float32, np.float32, 256, 256),
    "f32r": (mybir.dt.float32r, np.float32, 256, 256),
}

_built = {}
last_results = None


def _build(mode):
    if mode in _built:
        return _built[mode]
    cdt, _, pass_t, mm_n = _MODE_CFG[mode]
    n_pass = TOK_CORE // pass_t
    n_nt = pass_t // mm_n  # matmul n-tiles per pass

    nc = bacc.Bacc("TRN2", debug=False, num_devices=N_CORES)
    f32 = mybir.dt.float32

    xT = nc.dram_tensor("xT", [n_pass, P, KT * pass_t], cdt, kind="ExternalInput").ap()
    wgu = nc.dram_tensor("wgu", [MT, P, 2 * KT * P], cdt, kind="ExternalInput").ap()
    wd = nc.dram_tensor("wd", [KT, P, MT * P], cdt, kind="ExternalInput").ap()
    out = nc.dram_tensor("outT", [KT, P, TOK_CORE], f32, kind="ExternalOutput").ap()

    with tile.TileContext(nc) as tc, ExitStack() as ctx:
        xp = ctx.enter_context(tc.tile_pool(name="xp", bufs=1))
        wp = ctx.enter_context(tc.tile_pool(name="wp", bufs=3))
        wdp_ = ctx.enter_context(tc.tile_pool(name="wdp", bufs=2))
        hp = ctx.enter_context(tc.tile_pool(name="hp", bufs=1))
        sp = ctx.enter_context(tc.tile_pool(name="sp", bufs=2))
        op = ctx.enter_context(tc.tile_pool(name="op", bufs=3))
        ps = ctx.enter_context(tc.tile_pool(name="ps", bufs=2, space="PSUM"))

        for ip in range(n_pass):
            x_sb = xp.tile([P, KT, pass_t], cdt, tag="x")
            nc.sync.dma_start(out=x_sb, in_=xT[ip].rearrange("p (kt t) -> p kt t", kt=KT))
            h_sb = hp.tile([P, MT, pass_t], cdt, tag="h")

            # ---- gate/up + SiLU*mul ----
            for m in range(MT):
                w_sb = wp.tile([P, 2, KT, P], cdt, tag="wgu")
                nc.sync.dma_start(
                    out=w_sb, in_=wgu[m].rearrange("p (g kt q) -> p g kt q", g=2, kt=KT)
                )
                pg = ps.tile([P, pass_t], f32, tag="a")
                pu = ps.tile([P, pass_t], f32, tag="b")
                for k in range(KT):
                    for j in range(n_nt):
                        nc.tensor.matmul(
                            pg[:, j * mm_n:(j + 1) * mm_n], w_sb[:, 0, k, :],
                            x_sb[:, k, j * mm_n:(j + 1) * mm_n],
                            start=(k == 0), stop=(k == KT - 1),
                        )
                    for j in range(n_nt):
                        nc.tensor.matmul(
                            pu[:, j * mm_n:(j + 1) * mm_n], w_sb[:, 1, k, :],
                            x_sb[:, k, j * mm_n:(j + 1) * mm_n],
                            start=(k == 0), stop=(k == KT - 1),
                        )
                sg = sp.tile([P, pass_t], f32, tag="sg")
                nc.scalar.activation(sg, pg, mybir.ActivationFunctionType.Silu)
                nc.vector.tensor_mul(h_sb[:, m, :], sg, pu)

            # ---- down ----
            for m2 in range(KT):
                wd_sb = wdp_.tile([P, MT, P], cdt, tag="wd")
                nc.sync.dma_start(
                    out=wd_sb, in_=wd[m2].rearrange("p (kt q) -> p kt q", kt=MT)
                )
                pd = ps.tile([P, pass_t], f32, tag="a")
                for k2 in range(MT):
                    for j in range(n_nt):
                        nc.tensor.matmul(
                            pd[:, j * mm_n:(j + 1) * mm_n], wd_sb[:, k2, :],
                            h_sb[:, k2, j * mm_n:(j + 1) * mm_n],
                            start=(k2 == 0), stop=(k2 == MT - 1),
                        )
                o_sb = op.tile([P, pass_t], f32, tag="o")
                nc.vector.tensor_copy(o_sb, pd)
                nc.sync.dma_start(
                    out=out[m2][:, ip * pass_t:(ip + 1) * pass_t], in_=o_sb
                )

    nc.finalize()
    _built[mode] = nc
    return nc


def _materialize(w):
    """(q, p, b) circulant generators -> dense [p*b, q*b] (in-dim, out-dim)."""
    q, p, b = w.shape
    i = np.arange(b)
    idx = (i[None, :] - i[:, None]) % b          # [j, i]
    return w[:, :, idx].transpose(1, 2, 0, 3).reshape(p * b, q * b)


def kernel(x, w_gate, w_up, w_down):
    mode = MODE
    cdt, npdt, pass_t, mm_n = _MODE_CFG[mode]
    n_pass = TOK_CORE // pass_t

    nc = _build(mode)

    Wg = _materialize(np.asarray(w_gate, np.float32))   # [2048, 5632]
    Wu = _materialize(np.asarray(w_up, np.float32))     # [2048, 5632]
    Wd = _materialize(np.asarray(w_down, np.float32))   # [5632, 2048]

    # wgu packed: [MT, P, 2, KT, P]; per-partition rows contiguous
    wgu = np.empty((MT, P, 2, KT, P), np.float32)
    wg4 = Wg.reshape(KT, P, MT, P)   # [k, kp, m, mp]
    wu4 = Wu.reshape(KT, P, MT, P)
    wgu[:, :, 0] = wg4.transpose(2, 1, 0, 3)  # [m, kp, k, mp]
    wgu[:, :, 1] = wu4.transpose(2, 1, 0, 3)
    wgu = wgu.reshape(MT, P, 2 * KT * P).astype(npdt)

    wd4 = Wd.reshape(MT, P, KT, P)   # [k2, kp, m2, mp]
    wdp = np.ascontiguousarray(wd4.transpose(2, 1, 0, 3)).reshape(KT, P, MT * P).astype(npdt)

    xf = np.asarray(x, np.float32).reshape(TOK_TOTAL, D_MODEL)
    in_maps = []
    for c in range(N_CORES):
        xc = xf[c * TOK_CORE:(c + 1) * TOK_CORE]          # [2048 tok, 2048 d]
        # -> [n_pass, P, KT, pass_t]: xT[pass, kp, k, t] = xc[pass*pt+t, k*P+kp]
        xt = np.ascontiguousarray(
            xc.reshape(n_pass, pass_t, KT, P).transpose(0, 3, 2, 1)
        ).reshape(n_pass, P, KT * pass_t).astype(npdt)
        in_maps.append({"xT": xt, "wgu": wgu, "wd": wdp})

    trace = bool(os.environ.get("BASS_PROFILE"))
    res = run_bass_kernel_spmd(nc, in_maps, core_ids=list(range(N_CORES)), trace=trace)
    global last_results
    last_results = res

    out = np.empty((TOK_TOTAL, D_MODEL), np.float32)
    for c in range(N_CORES):
        o = res.results[c]["outT"]                         # [KT, P, TOK_CORE]
        out[c * TOK_CORE:(c + 1) * TOK_CORE] = o.reshape(D_MODEL, TOK_CORE).T
    return out.reshape(4, 4096, D_MODEL)
